# revision 23
# baseline (speedup 1.0000x reference)
"""Trainium2 Bass kernel for nn_MoEGraphLayer (moe_routing).

Sharding: 16 GAT units (4 graphs x {main, dep0, dep1, dep2}) over 8 cores,
2 units per core.  core c: batch b = c//2; half = c%2.
  half 0: unit a = main GAT,  unit b = deputy 0 (sentence)
  half 1: unit a = deputy 1 (section), unit b = deputy 2 (doc)
Each core also computes the blend gate for its batch; core outputs a partial
final output [S, D]; host adds the two halves per batch.

All attention math runs transposed (pT[t,s]) so the aggregation matmul needs
no on-device transposes of [S,S] tensors; adjacency is host-transposed.
Router top-2 mask is computed host-side (it is part of the sharding decision
and needs exact fp32 compare semantics vs the reference).
"""
import numpy as np
from contextlib import ExitStack

import concourse.bass as bass
import concourse.mybir as mybir
import concourse.tile as tile
from concourse import bacc
from concourse.bass_utils import run_bass_kernel_spmd
from concourse.masks import make_identity

F32 = mybir.dt.float32
F32R = mybir.dt.float32r
BF16 = mybir.dt.bfloat16
FP8 = mybir.dt.float8e4
AF = mybir.ActivationFunctionType
ALU = mybir.AluOpType

B, S, D, H, Hd, E = 4, 1024, 256, 6, 64, 3
NT = S // 128          # 8 t-chunks
ND = D // 128          # 2 d-chunks
NK = (H * Hd) // 128   # 3 k-chunks for W2
BIG = 60.0
TOP_K = 2
TARGET_MAIN_CONTRIBUTION = 0.6
CONTRIB_LOSS_COEF = 0.01

_NC_CACHE = {}


def _build_program():
    nc = bacc.Bacc("TRN2", target_bir_lowering=False, debug=False, num_devices=8)

    # ---------------- DRAM I/O ----------------
    xT_d = nc.dram_tensor("xT", [D, S], F32, kind="ExternalInput").ap()
    adjT_d = nc.dram_tensor("adjT", [S, S], F32, kind="ExternalInput").ap()
    blendW_d = nc.dram_tensor("blendW", [D, D], F32, kind="ExternalInput").ap()
    blendb_d = nc.dram_tensor("blendb", [D, 1], F32, kind="ExternalInput").ap()
    UW = {}
    for u in ("a", "b"):
        UW[u] = dict(
            W1cat=nc.dram_tensor(f"W1cat_{u}", [D, H * Hd + H], F32, kind="ExternalInput").ap(),
            W1s=nc.dram_tensor(f"W1s_{u}", [D, H], F32, kind="ExternalInput").ap(),
            W2=nc.dram_tensor(f"W2_{u}", [H * Hd, D], F32, kind="ExternalInput").ap(),
            w2sd=nc.dram_tensor(f"w2sd_{u}", [H * Hd, 2], F32, kind="ExternalInput").ap(),
            colmask=nc.dram_tensor(f"colmask_{u}", [128, NT], F32, kind="ExternalInput").ap(),
            rmask=nc.dram_tensor(f"rmask_{u}", [1, S], F32, kind="ExternalInput").ap(),
            wmask=nc.dram_tensor(f"wmask_{u}", [1, S], F32, kind="ExternalInput").ap(),
            alpha=nc.dram_tensor(f"alpha_{u}", [1, 1], F32, kind="ExternalInput").ap(),
        )
    out_d = nc.dram_tensor("out", [S, D], F32, kind="ExternalOutput").ap()
    bsum_d = nc.dram_tensor("bsum", [128, ND], F32, kind="ExternalOutput").ap()

    with tile.TileContext(nc) as tc, ExitStack() as ctx:
        # ---------------- pools ----------------
        wpool = ctx.enter_context(tc.tile_pool(name="weights", bufs=1))
        base = ctx.enter_context(tc.tile_pool(name="base", bufs=1))
        work = ctx.enter_context(tc.tile_pool(name="work", bufs=4))
        ppool = ctx.enter_context(tc.tile_pool(name="ppool", bufs=2))
        bcast = ctx.enter_context(tc.tile_pool(name="bcast", bufs=2))
        pairp = ctx.enter_context(tc.tile_pool(name="pair", bufs=2))
        combp = ctx.enter_context(tc.tile_pool(name="comb", bufs=3))
        osbp = ctx.enter_context(tc.tile_pool(name="osb", bufs=4))
        rowp = ctx.enter_context(tc.tile_pool(name="rows", bufs=1))
        normp = ctx.enter_context(tc.tile_pool(name="norm", bufs=3))
        ubuf = {u: ctx.enter_context(tc.tile_pool(name=f"ubuf_{u}", bufs=1)) for u in "ab"}
        acc = ctx.enter_context(tc.tile_pool(name="acc", bufs=2, space="PSUM"))
        tmp = ctx.enter_context(tc.tile_pool(name="tmp", bufs=2, space="PSUM"))
        dr = ctx.enter_context(tc.tile_pool(name="dram", bufs=1, space="DRAM"))

        const1 = nc.const_aps.tensor(1.0, (128, 1), F32)

        # ---------------- constants / setup ----------------
        ident = base.tile([128, 128], F32)
        make_identity(nc, ident[:])
        onesr = base.tile([128, 1], F32R)
        nc.vector.tensor_copy(onesr[:], const1)
        negbig = base.tile([128, 1], F32)
        nc.vector.tensor_scalar(negbig[:], in0=const1, scalar1=-BIG, scalar2=None,
                                op0=ALU.mult)

        # ---------------- feature load + per-unit masked copies ---------------
        xTrr = base.tile([128, ND, S], F32R, tag="xTrr")  # raw feature^T (f32 bits)
        for j in range(ND):
            nc.sync.dma_start(xTrr[:, j, :], xT_d[bass.ts(j, 128), :].bitcast(F32R))
        xTu = {}
        for u in ("a", "b"):
            rmb = bcast.tile([128, S], F32, tag="bcast")
            nc.sync.dma_start(rmb[:], UW[u]["rmask"].to_broadcast((128, S)))
            xTu[u] = base.tile([128, ND, S], F32R, tag=f"xTu_{u}", name=f"xTu_{u}")
            nc.vector.tensor_mul(xTu[u][:], xTrr[:].bitcast(F32),
                                 rmb[:, None, :].broadcast_to((128, ND, S)))

        # ---------------- weight loads (bitcast f32r where matmul-consumed) ----
        W1cat = {}; W1s = {}; W2 = {}; w2sd = {}; cmask = {}; alpha = {}
        for u in ("a", "b"):
            W1cat[u] = wpool.tile([128, ND, H * Hd + H], F32R, tag=f"w1c{u}", name=f"w1c{u}")
            W1s[u] = wpool.tile([128, ND, H], F32R, tag=f"w1s{u}", name=f"w1s{u}")
            W2[u] = wpool.tile([128, NK, D], F32R, tag=f"w2{u}", name=f"w2{u}")
            w2sd[u] = wpool.tile([128, NK, 2], F32R, tag=f"w2sd{u}", name=f"w2sd{u}")
            cmask[u] = wpool.tile([128, NT], F32, tag=f"cm{u}", name=f"cm{u}")
            alpha[u] = wpool.tile([128, 1], F32, tag=f"al{u}", name=f"al{u}")
            for j in range(ND):
                nc.sync.dma_start(W1s[u][:, j, :],
                                  UW[u]["W1s"][bass.ts(j, 128), :].bitcast(F32R))
            for j in range(ND):
                nc.sync.dma_start(W1cat[u][:, j, :],
                                  UW[u]["W1cat"][bass.ts(j, 128), :].bitcast(F32R))
            for k in range(NK):
                nc.sync.dma_start(W2[u][:, k, :],
                                  UW[u]["W2"][bass.ts(k, 128), :].bitcast(F32R))
                nc.sync.dma_start(w2sd[u][:, k, :],
                                  UW[u]["w2sd"][bass.ts(k, 128), :].bitcast(F32R))
            nc.sync.dma_start(cmask[u][:], UW[u]["colmask"][:])
            nc.sync.dma_start(alpha[u][:], UW[u]["alpha"].to_broadcast((128, 1)))
        blendW = wpool.tile([128, ND, D], F32R, tag="bw")
        bb = wpool.tile([128, ND], F32, tag="bb")
        for j in range(ND):
            nc.sync.dma_start(blendW[:, j, :],
                              blendW_d[bass.ts(j, 128), :].bitcast(F32R))
            nc.sync.dma_start(bb[:, j:j + 1], blendb_d[bass.ts(j, 128), :])

        # ---------------- blend gate (sigmoid) + combine weights ---------------
        # w_u[m] = alpha_u*blend + (1-blend)*wmask_u = alpha_u*blend - (blend-1)*wmask_u
        bs = base.tile([128, ND], F32)
        wmb = {}
        for u in ("a", "b"):
            wmb[u] = bcast.tile([128, S], F32, tag="bcast", name=f"wmb_{u}")
            nc.sync.dma_start(wmb[u][:], UW[u]["wmask"].to_broadcast((128, S)))
        w8 = {u: base.tile([128, ND, S], F32, tag=f"w8_{u}", name=f"w8_{u}") for u in "ab"}
        for m in range(ND):
            bp = tmp.tile([128, S], F32, tag="tmp", name="bp")
            for nh in range(2):
                for j in range(ND):
                    nc.tensor.matmul(
                        bp[:, bass.ts(nh, 512)],
                        blendW[:, j, bass.ts(m, 128)],
                        xTrr[:, j, bass.ts(nh, 512)],
                        start=(j == 0), stop=(j == ND - 1))
            blm = combp.tile([128, S], F32, tag="comb", name="blm")
            nc.scalar.activation(blm[:], bp[:], AF.Sigmoid,
                                 bias=bb[:, m:m + 1], scale=1.0,
                                 accum_out=bs[:, m:m + 1])
            for u in ("a", "b"):
                t_ = combp.tile([128, S], F32, tag="comb", name="t_")
                nc.vector.scalar_tensor_tensor(
                    t_[:], in0=blm[:], scalar=1.0, in1=wmb[u][:],
                    op0=ALU.subtract, op1=ALU.mult)
                nc.vector.scalar_tensor_tensor(
                    w8[u][:, m, :], in0=blm[:], scalar=alpha[u][:],
                    in1=t_[:], op0=ALU.mult, op1=ALU.subtract)
        nc.sync.dma_start(bsum_d[:], bs[:])

        # ---------------- per-unit GAT ----------------------------------------
        h65 = {}; ed = {}; h1e = {}; fg = {}; h2e = {}; outTn = {}
        esdr = {}; recdr = {}; fdr = {}
        for u in ("a", "b"):
            esdr[u] = dr.tile([H, S], F32, tag=f"esdr{u}", name=f"esdr{u}")
            recdr[u] = dr.tile([H, S], F32, tag=f"recdr{u}", name=f"recdr{u}")
            fdr[u] = dr.tile([1, S], F32, tag=f"fdr{u}", name=f"fdr{u}")

        def es_chain(u):
            # es rows via W1s -> psum [H, S] -> sbuf -> dram (early: the DRAM
            # roundtrip overlaps everything that follows)
            esp = acc.tile([H, S], F32, tag="acc", name="esp")
            for nh in range(2):
                for j in range(ND):
                    nc.tensor.matmul(esp[:, bass.ts(nh, 512)], W1s[u][:, j, :],
                                     xTu[u][:, j, bass.ts(nh, 512)],
                                     start=(j == 0), stop=(j == ND - 1))
            essb = pairp.tile([H, S], F32, tag="pair", name="essb")
            nc.vector.tensor_copy(essb[:], esp[:])
            nc.sync.dma_start(esdr[u][:], essb[:])

        def stage1(u):
            # h_all + ed via W1cat;  h65 layout: per head 65 cols = [ones | h]
            h65[u] = base.tile([128, NT, H * 65], F32R, tag=f"h65_{u}", name=f"h65_{u}")
            ed[u] = base.tile([128, NT, H], F32, tag=f"ed_{u}", name=f"ed_{u}")
            for t in range(NT):
                m1p = tmp.tile([128, H * Hd + H], F32, tag="tmp", name="m1p")
                for j in range(ND):
                    nc.tensor.matmul(m1p[:], xTu[u][:, j, bass.ts(t, 128)],
                                     W1cat[u][:, j, :],
                                     start=(j == 0), stop=(j == ND - 1))
                dst = h65[u][:, t, :].rearrange("p (g c) -> p g c", g=H)
                nc.vector.tensor_copy(
                    dst[:, :, 1:65],
                    m1p[:, 0:H * Hd].rearrange("p (g c) -> p g c", g=H))
                nc.vector.tensor_copy(dst[:, :, 0:1], const1.to_broadcast((128, H, 1)))
                nc.vector.tensor_copy(ed[u][:, t, :], m1p[:, H * Hd:H * Hd + H])

        es_chain("a")
        es_chain("b")
        stage1("a")

        # ---------------- adjacency: adjE_u = adjT * colmask_u (fp8) -----------
        adjE = {u: base.tile([128, NT, S], FP8, tag=f"adjE_{u}", name=f"adjE_{u}") for u in "ab"}
        for t in range(NT):
            atmp = work.tile([128, S], F32, tag="work", name="atmp")
            nc.sync.dma_start(atmp[:], adjT_d[bass.ts(t, 128), :])
            for u in ("a", "b"):
                nc.gpsimd.tensor_scalar(adjE[u][:, t, :], in0=atmp[:],
                                        scalar1=cmask[u][:, t:t + 1], scalar2=None,
                                        op0=ALU.mult)

        stage1("b")

        # ---- per-unit pipeline: L1 attention -> normalize+ELU -> W2 + f/g ----
        # Unit-sequential so unit b's ACT-heavy L1 overlaps unit a's DVE/PE tail.
        h1raw = {u: ubuf[u].tile([128, NK, S], F32, tag=f"ubuf_{u}", name=f"h1raw_{u}") for u in "ab"}

        def normalize_chunk(u, k):
            """h1e[:, k, :] = elu(h1raw[:, k, :] * recip_pair) as f32r."""
            pair = pairp.tile([128, S], F32, tag="pair", name="pair")
            nc.sync.dma_start(pair[0:64, :],
                              recdr[u][2 * k:2 * k + 1, :].to_broadcast((64, S)))
            nc.sync.dma_start(pair[64:128, :],
                              recdr[u][2 * k + 1:2 * k + 2, :].to_broadcast((64, S)))
            t1 = normp.tile([128, S], F32, tag="norm", name="t1")
            nc.vector.tensor_mul(t1[:], h1raw[u][:, k, :], pair[:])
            mn = normp.tile([128, S], F32, tag="norm", name="mn")
            nc.vector.tensor_scalar(mn[:], in0=t1[:], scalar1=0.0, scalar2=None,
                                    op0=ALU.min)
            em = normp.tile([128, S], F32, tag="norm", name="em")
            nc.scalar.activation(em[:], mn[:], AF.Exp)
            r = normp.tile([128, S], F32, tag="norm", name="r")
            nc.scalar.activation(r[:], t1[:], AF.Relu)
            nc.vector.scalar_tensor_tensor(
                h1e[u][:, k, :], in0=em[:], scalar=-1.0, in1=r[:],
                op0=ALU.add, op1=ALU.add)

        for u in ("a", "b"):
            h1e[u] = base.tile([128, NK, S], F32R, tag=f"xTu_{u}", name=f"h1e_{u}")
            for h in range(H):
                esb = bcast.tile([128, S], F32, tag="bcast", name="esb")
                nc.sync.dma_start(esb[:], esdr[u][h:h + 1, :].to_broadcast((128, S)))
                h1p = acc.tile([65, S], F32, tag="acc", name="h1p")
                for t in range(NT):
                    e = work.tile([128, S], F32, tag="work", name="e")
                    nc.scalar.activation(e[:], esb[:], AF.Prelu,
                                         bias=ed[u][:, t, h:h + 1], scale=1.0,
                                         alpha=0.2)
                    q = work.tile([128, S], F32, tag="work", name="q")
                    nc.vector.scalar_tensor_tensor(
                        q[:], in0=e[:], scalar=BIG, in1=adjE[u][:, t, :],
                        op0=ALU.add, op1=ALU.mult)
                    p = ppool.tile([128, S], F32R, tag="ppool", name="p")
                    nc.scalar.activation(p[:], q[:], AF.Exp, bias=negbig[:], scale=1.0)
                    for nh in range(2):
                        nc.tensor.matmul(
                            h1p[:, bass.ts(nh, 512)],
                            h65[u][:, t, h * 65:(h + 1) * 65],
                            p[:, bass.ts(nh, 512)],
                            start=(t == 0), stop=(t == NT - 1))
                # rowsum (row 0) -> recip -> dram; h rows (1:65) -> shift to h1raw
                rrow = rowp.tile([1, S], F32, tag="rows", name="rrow")
                nc.vector.reciprocal_approx_fast(rrow[:], h1p[0:1, :])
                nc.sync.dma_start(recdr[u][h:h + 1, :], rrow[:])
                stg = pairp.tile([65, S], F32, tag="pair", name="stg")
                nc.vector.tensor_copy(stg[:], h1p[:])
                nc.sync.dma_start(
                    h1raw[u][(h % 2) * 64:(h % 2) * 64 + 64, h // 2, :], stg[1:65, :])
                if h >= 2 and h % 2 == 0:
                    normalize_chunk(u, h // 2 - 1)
            normalize_chunk(u, 2)

            # W2 projection + f/g columns
            h2e[u] = ubuf[u].tile([128, NT, 2, 128], F32R, tag=f"ubuf_{u}", name=f"h2e_{u}")
            fg[u] = base.tile([128, NT, 2], F32, tag=f"fg_{u}", name=f"fg_{u}")
            for t in range(NT):
                w2p = tmp.tile([128, D], F32, tag="tmp", name="w2p")
                for k in range(NK):
                    nc.tensor.matmul(w2p[:], h1e[u][:, k, bass.ts(t, 128)],
                                     W2[u][:, k, :], start=(k == 0), stop=(k == NK - 1))
                nc.vector.tensor_copy(
                    h2e[u][:, t, :, :],
                    w2p[:].rearrange("p (m c) -> p m c", m=2))
                fgp = tmp.tile([128, 2], F32, tag="tmp", name="fgp")
                for k in range(NK):
                    nc.tensor.matmul(fgp[:], h1e[u][:, k, bass.ts(t, 128)],
                                     w2sd[u][:, k, :], start=(k == 0), stop=(k == NK - 1))
                nc.vector.tensor_copy(fg[u][:, t, :], fgp[:])
                nc.sync.dma_start(fdr[u][0:1, bass.ts(t, 128)], fg[u][:, t, 0:1])

        # ---- L2 attention + fused normalize*combine-weight -------------------
        for u in ("a", "b"):
            fb = bcast.tile([128, S], F32, tag="bcast", name="fb")
            nc.sync.dma_start(fb[:], fdr[u][:].to_broadcast((128, S)))
            aggA = acc.tile([128, S], F32, tag="acc", name="aggA")
            aggB = acc.tile([128, S], F32, tag="acc", name="aggB")
            rs2 = tmp.tile([1, S], F32, tag="tmp", name="rs2")
            for t in range(NT):
                e2 = work.tile([128, S], F32, tag="work", name="e2")
                nc.scalar.activation(e2[:], fb[:], AF.Prelu,
                                     bias=fg[u][:, t, 1:2], scale=1.0, alpha=0.2)
                q2 = work.tile([128, S], F32, tag="work", name="q2")
                nc.vector.scalar_tensor_tensor(
                    q2[:], in0=e2[:], scalar=BIG, in1=adjE[u][:, t, :],
                    op0=ALU.add, op1=ALU.mult)
                p2 = ppool.tile([128, S], F32R, tag="ppool", name="p2")
                nc.scalar.activation(p2[:], q2[:], AF.Exp, bias=negbig[:], scale=1.0)
                for nh in range(2):
                    nc.tensor.matmul(aggA[:, bass.ts(nh, 512)], h2e[u][:, t, 0, :],
                                     p2[:, bass.ts(nh, 512)],
                                     start=(t == 0), stop=(t == NT - 1))
                    nc.tensor.matmul(aggB[:, bass.ts(nh, 512)], h2e[u][:, t, 1, :],
                                     p2[:, bass.ts(nh, 512)],
                                     start=(t == 0), stop=(t == NT - 1))
                    nc.tensor.matmul(rs2[:, bass.ts(nh, 512)], onesr[:],
                                     p2[:, bass.ts(nh, 512)],
                                     start=(t == 0), stop=(t == NT - 1))
            r2row = rowp.tile([1, S], F32, tag="rows", name="r2row")
            nc.vector.reciprocal_approx_fast(r2row[:], rs2[:])
            r2b = pairp.tile([128, S], F32, tag="pair", name="r2b")
            nc.gpsimd.partition_broadcast(r2b[:], r2row[0:1, :])
            # unit a reuses the (long-dead) xTrr slot
            outTn[u] = base.tile([128, ND, S], F32, name=f"outTn_{u}",
                                 tag=("xTrr" if u == "a" else f"outTn_{u}"))
            for m, aggM in ((0, aggA), (1, aggB)):
                rw = combp.tile([128, S], F32, tag="comb", name="rw")
                nc.vector.tensor_mul(rw[:], r2b[:], w8[u][:, m, :])
                nc.vector.tensor_mul(outTn[u][:, m, :], aggM[:], rw[:])

        # ---- final add + transpose out (block-wise, pipelined) ----------------
        for m in range(ND):
            for sgm in range(NT):
                pb = combp.tile([128, 128], F32, tag="comb", name="pblk")
                nc.vector.tensor_add(pb[:],
                                     outTn["a"][:, m, bass.ts(sgm, 128)],
                                     outTn["b"][:, m, bass.ts(sgm, 128)])
                tp = tmp.tile([128, 128], F32, tag="tmp", name="tp")
                nc.tensor.transpose(tp[:], pb[:], ident[:])
                osb = osbp.tile([128, 128], F32, tag="osb", name="osb")
                nc.vector.tensor_copy(osb[:], tp[:])
                nc.sync.dma_start(
                    out_d[bass.ts(sgm, 128), bass.ts(m, 128)], osb[:])

    nc.compile()
    return nc


def _routing_masks(feature, router_W):
    """Top-2-of-3 routing masks, replicating jax.lax.top_k tie semantics."""
    x = feature.reshape(-1, D).astype(np.float32)
    logits = x @ router_W.astype(np.float32)
    m = logits.max(axis=-1, keepdims=True)
    ex = np.exp(logits - m, dtype=np.float32)
    g = ex / ex.sum(axis=-1, keepdims=True, dtype=np.float32)
    g0, g1, g2 = g[:, 0], g[:, 1], g[:, 2]
    drop2 = (g2 <= g0) & (g2 <= g1)
    drop1 = ~drop2 & (g1 <= g0) & (g1 <= g2)
    drop0 = ~(drop1 | drop2)
    mask = np.stack([~drop0, ~drop1, ~drop2], axis=-1).astype(np.float32)
    return mask.reshape(B, S, E)


def kernel(feature, adj, main_W1, main_a1s, main_a1d, main_W2, main_a2s, main_a2d,
           dep_W1, dep_a1s, dep_a1d, dep_W2, dep_a2s, dep_a2d,
           router_W, blend_W, blend_b, doc_num, sect_num):
    feature = np.asarray(feature, dtype=np.float32)
    adj = np.asarray(adj, dtype=np.float32)
    doc_num = int(doc_num)
    sect_num = int(sect_num)

    if "nc" not in _NC_CACHE:
        _NC_CACHE["nc"] = _build_program()
    nc = _NC_CACHE["nc"]

    # ---- host-side prep (sharding + weight packing) ----
    col = np.arange(S)
    colmasks = [
        (col < S - sect_num - doc_num).astype(np.float32),
        ((col >= S - sect_num - doc_num) & (col < S - doc_num)).astype(np.float32),
        (col >= S - doc_num).astype(np.float32),
    ]
    rmask = _routing_masks(feature, router_W)      # [B, S, E]
    onesS = np.ones(S, np.float32)

    def pack_unit(expert):
        """expert: -1 = main, 0..2 = deputy index. Returns weight dict pieces."""
        if expert < 0:
            W1, a1s, a1d = main_W1, main_a1s, main_a1d
            W2_, a2s, a2d = main_W2, main_a2s, main_a2d
            cm = onesS
        else:
            W1, a1s, a1d = dep_W1[expert], dep_a1s[expert], dep_a1d[expert]
            W2_, a2s, a2d = dep_W2[expert], dep_a2s[expert], dep_a2d[expert]
            cm = colmasks[expert]
        W1 = np.asarray(W1, np.float32); W2_ = np.asarray(W2_, np.float32)
        a1s = np.asarray(a1s, np.float32); a1d = np.asarray(a1d, np.float32)
        a2s = np.asarray(a2s, np.float32); a2d = np.asarray(a2d, np.float32)
        W1cat = np.concatenate(
            [W1.transpose(1, 0, 2).reshape(D, H * Hd),
             np.einsum("hdf,hf->dh", W1, a1d)], axis=1)          # [D, 390]
        W1s_ = np.einsum("hdf,hf->dh", W1, a1s)                   # [D, 6]
        w2sd = np.stack([W2_ @ a2s, W2_ @ a2d], axis=1)           # [384, 2]
        return dict(W1cat=np.ascontiguousarray(W1cat),
                    W1s=np.ascontiguousarray(W1s_),
                    W2=np.ascontiguousarray(W2_),
                    w2sd=np.ascontiguousarray(w2sd),
                    colmask=np.ascontiguousarray(cm.reshape(NT, 128).T))

    unit_specs = {0: (-1, 0), 1: (1, 2)}  # half -> (expert_a, expert_b)
    packs = {e: pack_unit(e) for e in (-1, 0, 1, 2)}
    blend_W = np.ascontiguousarray(np.asarray(blend_W, np.float32))
    blend_b = np.ascontiguousarray(np.asarray(blend_b, np.float32).reshape(D, 1))

    in_maps = []
    for c in range(8):
        b, half = c // 2, c % 2
        ea, eb = unit_specs[half]
        m = {
            "xT": np.ascontiguousarray(feature[b].T),
            "adjT": np.ascontiguousarray(adj[b].T),
            "blendW": blend_W, "blendb": blend_b,
        }
        for u, e in (("a", ea), ("b", eb)):
            pk = packs[e]
            m[f"W1cat_{u}"] = pk["W1cat"]; m[f"W1s_{u}"] = pk["W1s"]
            m[f"W2_{u}"] = pk["W2"]; m[f"w2sd_{u}"] = pk["w2sd"]
            m[f"colmask_{u}"] = pk["colmask"]
            if e < 0:
                rm = onesS; wm = np.zeros(S, np.float32); al = 1.0
            else:
                rm = rmask[b, :, e]; wm = rm; al = 0.0
            m[f"rmask_{u}"] = np.ascontiguousarray(rm.reshape(1, S))
            m[f"wmask_{u}"] = np.ascontiguousarray(wm.reshape(1, S))
            m[f"alpha_{u}"] = np.array([[al]], np.float32)
        in_maps.append(m)

    res = run_bass_kernel_spmd(nc, in_maps, core_ids=list(range(8)))

    final = np.empty((B, S, D), np.float32)
    bsum_total = np.float32(0.0)
    for b in range(B):
        final[b] = res.results[2 * b]["out"] + res.results[2 * b + 1]["out"]
        bsum_total += res.results[2 * b]["bsum"].sum(dtype=np.float32)
    mc = np.float32(bsum_total / (B * S * D))
    loss = np.float32(abs(mc - np.float32(TARGET_MAIN_CONTRIBUTION))
                      * np.float32(CONTRIB_LOSS_COEF))
    return final, loss, mc


# revision 26
# speedup vs baseline: 1.0542x; 1.0542x over previous
"""Trainium2 Bass kernel for nn_MoEGraphLayer (moe_routing).

Sharding: 16 GAT units (4 graphs x {main, dep0, dep1, dep2}) over 8 cores,
2 units per core.  core c: batch b = c//2; half = c%2.
  half 0: unit a = main GAT,  unit b = deputy 0 (sentence)
  half 1: unit a = deputy 1 (section), unit b = deputy 2 (doc)
Each core also computes the blend gate for its batch; core outputs a partial
final output [S, D]; host adds the two halves per batch.

All attention math runs transposed (pT[t,s]) so the aggregation matmul needs
no on-device transposes of [S,S] tensors; adjacency is host-transposed.
Router top-2 mask is computed host-side (it is part of the sharding decision
and needs exact fp32 compare semantics vs the reference).
"""
import numpy as np
from contextlib import ExitStack

import concourse.bass as bass
import concourse.mybir as mybir
import concourse.tile as tile
from concourse import bacc
from concourse.bass_utils import run_bass_kernel_spmd
from concourse.masks import make_identity

F32 = mybir.dt.float32
F32R = mybir.dt.float32r
BF16 = mybir.dt.bfloat16
FP8 = mybir.dt.float8e4
AF = mybir.ActivationFunctionType
ALU = mybir.AluOpType

B, S, D, H, Hd, E = 4, 1024, 256, 6, 64, 3
NT = S // 128          # 8 t-chunks
ND = D // 128          # 2 d-chunks
NK = (H * Hd) // 128   # 3 k-chunks for W2
BIG = 60.0
TOP_K = 2
TARGET_MAIN_CONTRIBUTION = 0.6
CONTRIB_LOSS_COEF = 0.01

_NC_CACHE = {}


def _build_program():
    nc = bacc.Bacc("TRN2", target_bir_lowering=False, debug=False, num_devices=8)

    # ---------------- DRAM I/O ----------------
    xT_d = nc.dram_tensor("xT", [D, S], F32, kind="ExternalInput").ap()
    adjT_d = nc.dram_tensor("adjT", [S, S], F32, kind="ExternalInput").ap()
    blendW_d = nc.dram_tensor("blendW", [D, D], F32, kind="ExternalInput").ap()
    blendb_d = nc.dram_tensor("blendb", [D, 1], F32, kind="ExternalInput").ap()
    UW = {}
    for u in ("a", "b"):
        UW[u] = dict(
            W1cat=nc.dram_tensor(f"W1cat_{u}", [D, H * Hd + H], F32, kind="ExternalInput").ap(),
            W1s=nc.dram_tensor(f"W1s_{u}", [D, H], F32, kind="ExternalInput").ap(),
            W2=nc.dram_tensor(f"W2_{u}", [H * Hd, D], F32, kind="ExternalInput").ap(),
            w2sd=nc.dram_tensor(f"w2sd_{u}", [H * Hd, 2], F32, kind="ExternalInput").ap(),
            colmask=nc.dram_tensor(f"colmask_{u}", [128, NT], F32, kind="ExternalInput").ap(),
            rmask=nc.dram_tensor(f"rmask_{u}", [1, S], F32, kind="ExternalInput").ap(),
            wmask=nc.dram_tensor(f"wmask_{u}", [1, S], F32, kind="ExternalInput").ap(),
            alpha=nc.dram_tensor(f"alpha_{u}", [1, 1], F32, kind="ExternalInput").ap(),
        )
    out_d = nc.dram_tensor("out", [S, D], F32, kind="ExternalOutput").ap()
    bsum_d = nc.dram_tensor("bsum", [128, ND], F32, kind="ExternalOutput").ap()

    with tile.TileContext(nc) as tc, ExitStack() as ctx:
        # ---------------- pools ----------------
        wpool = ctx.enter_context(tc.tile_pool(name="weights", bufs=1))
        base = ctx.enter_context(tc.tile_pool(name="base", bufs=1))
        work = ctx.enter_context(tc.tile_pool(name="work", bufs=3))
        ppool = ctx.enter_context(tc.tile_pool(name="ppool", bufs=3))
        bcast = ctx.enter_context(tc.tile_pool(name="bcast", bufs=2))
        pairp = ctx.enter_context(tc.tile_pool(name="pair", bufs=2))
        combp = ctx.enter_context(tc.tile_pool(name="comb", bufs=3))
        osbp = ctx.enter_context(tc.tile_pool(name="osb", bufs=4))
        rowp = ctx.enter_context(tc.tile_pool(name="rows", bufs=1))
        normp = ctx.enter_context(tc.tile_pool(name="norm", bufs=3))
        qpair = ctx.enter_context(tc.tile_pool(name="qpair", bufs=2))
        ubuf = {u: ctx.enter_context(tc.tile_pool(name=f"ubuf_{u}", bufs=1)) for u in "ab"}
        acc = ctx.enter_context(tc.tile_pool(name="acc", bufs=2, space="PSUM"))
        tmp = ctx.enter_context(tc.tile_pool(name="tmp", bufs=2, space="PSUM"))
        dr = ctx.enter_context(tc.tile_pool(name="dram", bufs=1, space="DRAM"))

        const1 = nc.const_aps.tensor(1.0, (128, 1), F32)

        # ---------------- constants / setup ----------------
        ident = base.tile([128, 128], F32)
        make_identity(nc, ident[:])
        onesr = base.tile([128, 1], BF16)
        nc.vector.tensor_copy(onesr[:], const1)
        negbig = base.tile([128, 1], F32)
        nc.vector.tensor_scalar(negbig[:], in0=const1, scalar1=-BIG, scalar2=None,
                                op0=ALU.mult)

        # ---------------- loads + setup, ordered by urgency ---------------
        # urgency: xT -> rmask/xTu -> W1s -> es chains -> cm/adjE -> W1cat ->
        # stage1 -> blend weights -> blend/w8 -> W2/w2sd (late)
        xTrr = base.tile([128, ND, S], F32R, tag="xTrr")  # raw feature^T (f32 bits)
        for j in range(ND):
            nc.sync.dma_start(xTrr[:, j, :], xT_d[bass.ts(j, 128), :].bitcast(F32R))

        W1cat = {}; W1s = {}; W2 = {}; w2sd = {}; cmask = {}; alpha = {}
        h65 = {}; ed = {}; h1e = {}; fg = {}; h2e = {}; outTn = {}
        esdr = {}; recdr = {}; fdr = {}
        for u in ("a", "b"):
            esdr[u] = dr.tile([H, S], F32, tag=f"esdr{u}", name=f"esdr{u}")
            recdr[u] = dr.tile([H, S], F32, tag=f"recdr{u}", name=f"recdr{u}")
            fdr[u] = dr.tile([1, S], F32, tag=f"fdr{u}", name=f"fdr{u}")

        xTu = {}
        for u in ("a", "b"):
            rmb = bcast.tile([128, S], F32, tag="bcast")
            nc.sync.dma_start(rmb[:], UW[u]["rmask"].to_broadcast((128, S)))
            xTu[u] = base.tile([128, ND, S], F32R, tag=f"xTu_{u}", name=f"xTu_{u}")
            nc.vector.tensor_mul(xTu[u][:], xTrr[:].bitcast(F32),
                                 rmb[:, None, :].broadcast_to((128, ND, S)))
            W1s[u] = wpool.tile([128, ND, H], F32R, tag=f"w1s{u}", name=f"w1s{u}")
            for j in range(ND):
                nc.sync.dma_start(W1s[u][:, j, :],
                                  UW[u]["W1s"][bass.ts(j, 128), :].bitcast(F32R))
            # es rows via W1s -> psum [H, S] -> sbuf -> dram (early: the DRAM
            # roundtrip overlaps everything that follows)
            esp = acc.tile([H, S], F32, tag="acc", name="esp")
            for nh in range(2):
                for j in range(ND):
                    nc.tensor.matmul(esp[:, bass.ts(nh, 512)], W1s[u][:, j, :],
                                     xTu[u][:, j, bass.ts(nh, 512)],
                                     start=(j == 0), stop=(j == ND - 1))
            essb = pairp.tile([H, S], F32, tag="pair", name="essb")
            nc.vector.tensor_copy(essb[:], esp[:])
            nc.sync.dma_start(esdr[u][:], essb[:])

        # adjacency (needed by L1(a) tau 0 early)
        for u in ("a", "b"):
            cmask[u] = wpool.tile([128, NT], F32, tag=f"cm{u}", name=f"cm{u}")
            nc.sync.dma_start(cmask[u][:], UW[u]["colmask"][:])
        W1cat["a"] = wpool.tile([128, ND, H * Hd + H], F32R, tag="w1ca", name="w1ca")
        for j in range(ND):
            nc.sync.dma_start(W1cat["a"][:, j, :],
                              UW["a"]["W1cat"][bass.ts(j, 128), :].bitcast(F32R))
        adjE = {u: base.tile([128, NT, S], FP8, tag=f"adjE_{u}", name=f"adjE_{u}") for u in "ab"}
        for t in range(NT):
            atmp = work.tile([128, S], F32, tag="work", name="atmp")
            nc.sync.dma_start(atmp[:], adjT_d[bass.ts(t, 128), :])
            for u in ("a", "b"):
                nc.gpsimd.tensor_scalar(adjE[u][:, t, :], in0=atmp[:],
                                        scalar1=cmask[u][:, t:t + 1], scalar2=None,
                                        op0=ALU.mult)
        W1cat["b"] = wpool.tile([128, ND, H * Hd + H], F32R, tag="w1cb", name="w1cb")
        for j in range(ND):
            nc.sync.dma_start(W1cat["b"][:, j, :],
                              UW["b"]["W1cat"][bass.ts(j, 128), :].bitcast(F32R))

        def stage1(u):
            # h_all + ed via W1cat;  h65 layout: per head 65 cols = [ones | h]
            h65[u] = base.tile([128, NT, H * 65], BF16, tag=f"h65_{u}", name=f"h65_{u}")
            ed[u] = base.tile([128, NT, H], F32, tag=f"ed_{u}", name=f"ed_{u}")
            for t in range(NT):
                m1p = tmp.tile([128, H * Hd + H], F32, tag="tmp", name="m1p")
                for j in range(ND):
                    nc.tensor.matmul(m1p[:], xTu[u][:, j, bass.ts(t, 128)],
                                     W1cat[u][:, j, :],
                                     start=(j == 0), stop=(j == ND - 1))
                dst = h65[u][:, t, :].rearrange("p (g c) -> p g c", g=H)
                nc.vector.tensor_copy(
                    dst[:, :, 1:65],
                    m1p[:, 0:H * Hd].rearrange("p (g c) -> p g c", g=H))
                nc.vector.tensor_copy(dst[:, :, 0:1], const1.to_broadcast((128, H, 1)))
                nc.vector.tensor_copy(ed[u][:, t, :], m1p[:, H * Hd:H * Hd + H])

        stage1("a")

        # blend gate + combine weights (off the early critical path)
        blendW = wpool.tile([128, ND, D], F32R, tag="bw")
        bb = wpool.tile([128, ND], F32, tag="bb")
        for j in range(ND):
            nc.sync.dma_start(blendW[:, j, :],
                              blendW_d[bass.ts(j, 128), :].bitcast(F32R))
            nc.sync.dma_start(bb[:, j:j + 1], blendb_d[bass.ts(j, 128), :])
        bs = base.tile([128, ND], F32)
        wmb = {}
        for u in ("a", "b"):
            alpha[u] = wpool.tile([128, 1], F32, tag=f"al{u}", name=f"al{u}")
            nc.sync.dma_start(alpha[u][:], UW[u]["alpha"].to_broadcast((128, 1)))
            wmb[u] = bcast.tile([128, S], F32, tag="bcast", name=f"wmb_{u}")
            nc.sync.dma_start(wmb[u][:], UW[u]["wmask"].to_broadcast((128, S)))
        w8 = {u: base.tile([128, ND, S], BF16, tag=f"w8_{u}", name=f"w8_{u}") for u in "ab"}
        for m in range(ND):
            bp = tmp.tile([128, S], F32, tag="tmp", name="bp")
            for nh in range(2):
                for j in range(ND):
                    nc.tensor.matmul(
                        bp[:, bass.ts(nh, 512)],
                        blendW[:, j, bass.ts(m, 128)],
                        xTrr[:, j, bass.ts(nh, 512)],
                        start=(j == 0), stop=(j == ND - 1))
            blm = combp.tile([128, S], F32, tag="comb", name="blm")
            nc.scalar.activation(blm[:], bp[:], AF.Sigmoid,
                                 bias=bb[:, m:m + 1], scale=1.0,
                                 accum_out=bs[:, m:m + 1])
            for u in ("a", "b"):
                t_ = combp.tile([128, S], F32, tag="comb", name="t_")
                nc.vector.scalar_tensor_tensor(
                    t_[:], in0=blm[:], scalar=1.0, in1=wmb[u][:],
                    op0=ALU.subtract, op1=ALU.mult)
                nc.vector.scalar_tensor_tensor(
                    w8[u][:, m, :], in0=blm[:], scalar=alpha[u][:],
                    in1=t_[:], op0=ALU.mult, op1=ALU.subtract)
        nc.sync.dma_start(bsum_d[:], bs[:])

        stage1("b")

        # late weights: W2 / w2sd (first needed after L1)
        for u in ("a", "b"):
            W2[u] = wpool.tile([128, NK, D], F32R, tag=f"w2{u}", name=f"w2{u}")
            w2sd[u] = wpool.tile([128, NK, 2], F32R, tag=f"w2sd{u}", name=f"w2sd{u}")
            for k in range(NK):
                nc.sync.dma_start(W2[u][:, k, :],
                                  UW[u]["W2"][bass.ts(k, 128), :].bitcast(F32R))
                nc.sync.dma_start(w2sd[u][:, k, :],
                                  UW[u]["w2sd"][bass.ts(k, 128), :].bitcast(F32R))

        # ---- per-unit pipeline: L1 attention -> normalize+ELU -> W2 + f/g ----
        # Unit-sequential so unit b's ACT-heavy L1 overlaps unit a's DVE/PE tail.
        h1raw = {u: ubuf[u].tile([128, NK, S], F32, tag=f"ubuf_{u}", name=f"h1raw_{u}") for u in "ab"}

        def normalize_chunk(u, k):
            """h1e[:, k, :] = elu(h1raw[:, k, :] * recip_pair) as f32r."""
            pair = pairp.tile([128, S], F32, tag="pair", name="pair")
            nc.sync.dma_start(pair[0:64, :],
                              recdr[u][2 * k:2 * k + 1, :].to_broadcast((64, S)))
            nc.sync.dma_start(pair[64:128, :],
                              recdr[u][2 * k + 1:2 * k + 2, :].to_broadcast((64, S)))
            t1 = normp.tile([128, S], F32, tag="norm", name="t1")
            nc.gpsimd.tensor_tensor(t1[:], in0=h1raw[u][:, k, :], in1=pair[:],
                                    op=ALU.mult)
            mn = normp.tile([128, S], F32, tag="norm", name="mn")
            nc.vector.tensor_scalar(mn[:], in0=t1[:], scalar1=0.0, scalar2=None,
                                    op0=ALU.min)
            em = normp.tile([128, S], F32, tag="norm", name="em")
            nc.scalar.activation(em[:], mn[:], AF.Exp)
            r = normp.tile([128, S], F32, tag="norm", name="r")
            nc.scalar.activation(r[:], t1[:], AF.Relu)
            nc.vector.scalar_tensor_tensor(
                h1e[u][:, k, :], in0=em[:], scalar=-1.0, in1=r[:],
                op0=ALU.add, op1=ALU.add)

        for u in ("a", "b"):
            h1e[u] = base.tile([128, NK, S], F32R, tag=f"xTu_{u}", name=f"h1e_{u}")
            for hp in range(H // 2):
                h0 = 2 * hp
                esb = {}
                for g in (0, 1):
                    esb[g] = bcast.tile([128, S], F32, tag="bcast", name="esb")
                    nc.sync.dma_start(
                        esb[g][:], esdr[u][h0 + g:h0 + g + 1, :].to_broadcast((128, S)))
                h1p = {g: acc.tile([65, S], F32, tag="acc", name="h1p") for g in (0, 1)}
                for t in range(NT):
                    qp = qpair.tile([128, 2, S], F32, tag="qpair", name="qp")
                    for g in (0, 1):
                        e = work.tile([128, S], F32, tag="work", name="e")
                        nc.scalar.activation(e[:], esb[g][:], AF.Prelu,
                                             bias=ed[u][:, t, h0 + g:h0 + g + 1],
                                             scale=1.0, alpha=0.2)
                        nc.vector.scalar_tensor_tensor(
                            qp[:, g, :], in0=e[:], scalar=BIG, in1=adjE[u][:, t, :],
                            op0=ALU.add, op1=ALU.mult)
                    pp = ppool.tile([128, 2, S], BF16, tag="ppool", name="pp")
                    nc.scalar.activation(pp[:], qp[:], AF.Exp, bias=negbig[:], scale=1.0)
                    for g in (0, 1):
                        for nh in range(2):
                            nc.tensor.matmul(
                                h1p[g][:, bass.ts(nh, 512)],
                                h65[u][:, t, (h0 + g) * 65:(h0 + g + 1) * 65],
                                pp[:, g, bass.ts(nh, 512)],
                                start=(t == 0), stop=(t == NT - 1))
                for g in (0, 1):
                    # rowsum (row 0) -> recip -> dram; h rows (1:65) -> h1raw
                    rrow = rowp.tile([1, S], F32, tag="rows", name="rrow")
                    nc.vector.reciprocal_approx_fast(rrow[:], h1p[g][0:1, :])
                    nc.sync.dma_start(recdr[u][h0 + g:h0 + g + 1, :], rrow[:])
                    stg = pairp.tile([65, S], F32, tag="pair", name="stg")
                    nc.vector.tensor_copy(stg[:], h1p[g][:])
                    nc.sync.dma_start(
                        h1raw[u][g * 64:g * 64 + 64, hp, :], stg[1:65, :])
                if hp >= 1:
                    normalize_chunk(u, hp - 1)
            normalize_chunk(u, 2)

            # W2 projection + f/g columns
            h2e[u] = ubuf[u].tile([128, NT, 2, 128], BF16, tag=f"ubuf_{u}", name=f"h2e_{u}")
            fg[u] = base.tile([128, NT, 2], F32, tag=f"fg_{u}", name=f"fg_{u}")
            for t in range(NT):
                w2p = tmp.tile([128, D], F32, tag="tmp", name="w2p")
                for k in range(NK):
                    nc.tensor.matmul(w2p[:], h1e[u][:, k, bass.ts(t, 128)],
                                     W2[u][:, k, :], start=(k == 0), stop=(k == NK - 1))
                nc.vector.tensor_copy(
                    h2e[u][:, t, :, :],
                    w2p[:].rearrange("p (m c) -> p m c", m=2))
                fgp = tmp.tile([128, 2], F32, tag="tmp", name="fgp")
                for k in range(NK):
                    nc.tensor.matmul(fgp[:], h1e[u][:, k, bass.ts(t, 128)],
                                     w2sd[u][:, k, :], start=(k == 0), stop=(k == NK - 1))
                nc.vector.tensor_copy(fg[u][:, t, :], fgp[:])
                nc.sync.dma_start(fdr[u][0:1, bass.ts(t, 128)], fg[u][:, t, 0:1])

        # ---- L2 attention + fused normalize*combine-weight -------------------
        for u in ("a", "b"):
            fb = bcast.tile([128, S], F32, tag="bcast", name="fb")
            nc.sync.dma_start(fb[:], fdr[u][:].to_broadcast((128, S)))
            aggA = acc.tile([128, S], F32, tag="acc", name="aggA")
            aggB = acc.tile([128, S], F32, tag="acc", name="aggB")
            rs2 = tmp.tile([1, S], F32, tag="tmp", name="rs2")
            for t in range(NT):
                e2 = work.tile([128, S], F32, tag="work", name="e2")
                nc.scalar.activation(e2[:], fb[:], AF.Prelu,
                                     bias=fg[u][:, t, 1:2], scale=1.0, alpha=0.2)
                q2 = work.tile([128, S], F32, tag="work", name="q2")
                nc.vector.scalar_tensor_tensor(
                    q2[:], in0=e2[:], scalar=BIG, in1=adjE[u][:, t, :],
                    op0=ALU.add, op1=ALU.mult)
                p2 = ppool.tile([128, S], BF16, tag="ppool", name="p2")
                nc.scalar.activation(p2[:], q2[:], AF.Exp, bias=negbig[:], scale=1.0)
                for nh in range(2):
                    nc.tensor.matmul(aggA[:, bass.ts(nh, 512)], h2e[u][:, t, 0, :],
                                     p2[:, bass.ts(nh, 512)],
                                     start=(t == 0), stop=(t == NT - 1))
                    nc.tensor.matmul(aggB[:, bass.ts(nh, 512)], h2e[u][:, t, 1, :],
                                     p2[:, bass.ts(nh, 512)],
                                     start=(t == 0), stop=(t == NT - 1))
                    nc.tensor.matmul(rs2[:, bass.ts(nh, 512)], onesr[:],
                                     p2[:, bass.ts(nh, 512)],
                                     start=(t == 0), stop=(t == NT - 1))
            r2row = rowp.tile([1, S], F32, tag="rows", name="r2row")
            nc.vector.reciprocal_approx_fast(r2row[:], rs2[:])
            r2b = pairp.tile([128, S], F32, tag="pair", name="r2b")
            nc.gpsimd.partition_broadcast(r2b[:], r2row[0:1, :])
            # unit a reuses the (long-dead) xTrr slot
            outTn[u] = base.tile([128, ND, S], F32, name=f"outTn_{u}",
                                 tag=("xTrr" if u == "a" else f"outTn_{u}"))
            for m, aggM in ((0, aggA), (1, aggB)):
                rw = combp.tile([128, S], F32, tag="comb", name="rw")
                nc.vector.tensor_mul(rw[:], r2b[:], w8[u][:, m, :])
                nc.vector.tensor_mul(outTn[u][:, m, :], aggM[:], rw[:])

        # ---- final add + transpose out (block-wise, pipelined) ----------------
        for m in range(ND):
            for sgm in range(NT):
                pb = combp.tile([128, 128], F32, tag="comb", name="pblk")
                nc.vector.tensor_add(pb[:],
                                     outTn["a"][:, m, bass.ts(sgm, 128)],
                                     outTn["b"][:, m, bass.ts(sgm, 128)])
                tp = tmp.tile([128, 128], F32, tag="tmp", name="tp")
                nc.tensor.transpose(tp[:], pb[:], ident[:])
                osb = osbp.tile([128, 128], F32, tag="osb", name="osb")
                nc.vector.tensor_copy(osb[:], tp[:])
                nc.sync.dma_start(
                    out_d[bass.ts(sgm, 128), bass.ts(m, 128)], osb[:])

    nc.compile()
    return nc


def _routing_masks(feature, router_W):
    """Top-2-of-3 routing masks, replicating jax.lax.top_k tie semantics."""
    x = feature.reshape(-1, D).astype(np.float32)
    logits = x @ router_W.astype(np.float32)
    m = logits.max(axis=-1, keepdims=True)
    ex = np.exp(logits - m, dtype=np.float32)
    g = ex / ex.sum(axis=-1, keepdims=True, dtype=np.float32)
    g0, g1, g2 = g[:, 0], g[:, 1], g[:, 2]
    drop2 = (g2 <= g0) & (g2 <= g1)
    drop1 = ~drop2 & (g1 <= g0) & (g1 <= g2)
    drop0 = ~(drop1 | drop2)
    mask = np.stack([~drop0, ~drop1, ~drop2], axis=-1).astype(np.float32)
    return mask.reshape(B, S, E)


def kernel(feature, adj, main_W1, main_a1s, main_a1d, main_W2, main_a2s, main_a2d,
           dep_W1, dep_a1s, dep_a1d, dep_W2, dep_a2s, dep_a2d,
           router_W, blend_W, blend_b, doc_num, sect_num):
    feature = np.asarray(feature, dtype=np.float32)
    adj = np.asarray(adj, dtype=np.float32)
    doc_num = int(doc_num)
    sect_num = int(sect_num)

    if "nc" not in _NC_CACHE:
        _NC_CACHE["nc"] = _build_program()
    nc = _NC_CACHE["nc"]

    # ---- host-side prep (sharding + weight packing) ----
    col = np.arange(S)
    colmasks = [
        (col < S - sect_num - doc_num).astype(np.float32),
        ((col >= S - sect_num - doc_num) & (col < S - doc_num)).astype(np.float32),
        (col >= S - doc_num).astype(np.float32),
    ]
    rmask = _routing_masks(feature, router_W)      # [B, S, E]
    onesS = np.ones(S, np.float32)

    def pack_unit(expert):
        """expert: -1 = main, 0..2 = deputy index. Returns weight dict pieces."""
        if expert < 0:
            W1, a1s, a1d = main_W1, main_a1s, main_a1d
            W2_, a2s, a2d = main_W2, main_a2s, main_a2d
            cm = onesS
        else:
            W1, a1s, a1d = dep_W1[expert], dep_a1s[expert], dep_a1d[expert]
            W2_, a2s, a2d = dep_W2[expert], dep_a2s[expert], dep_a2d[expert]
            cm = colmasks[expert]
        W1 = np.asarray(W1, np.float32); W2_ = np.asarray(W2_, np.float32)
        a1s = np.asarray(a1s, np.float32); a1d = np.asarray(a1d, np.float32)
        a2s = np.asarray(a2s, np.float32); a2d = np.asarray(a2d, np.float32)
        W1cat = np.concatenate(
            [W1.transpose(1, 0, 2).reshape(D, H * Hd),
             np.einsum("hdf,hf->dh", W1, a1d)], axis=1)          # [D, 390]
        W1s_ = np.einsum("hdf,hf->dh", W1, a1s)                   # [D, 6]
        w2sd = np.stack([W2_ @ a2s, W2_ @ a2d], axis=1)           # [384, 2]
        return dict(W1cat=np.ascontiguousarray(W1cat),
                    W1s=np.ascontiguousarray(W1s_),
                    W2=np.ascontiguousarray(W2_),
                    w2sd=np.ascontiguousarray(w2sd),
                    colmask=np.ascontiguousarray(cm.reshape(NT, 128).T))

    unit_specs = {0: (-1, 0), 1: (1, 2)}  # half -> (expert_a, expert_b)
    packs = {e: pack_unit(e) for e in (-1, 0, 1, 2)}
    blend_W = np.ascontiguousarray(np.asarray(blend_W, np.float32))
    blend_b = np.ascontiguousarray(np.asarray(blend_b, np.float32).reshape(D, 1))

    in_maps = []
    for c in range(8):
        b, half = c // 2, c % 2
        ea, eb = unit_specs[half]
        m = {
            "xT": np.ascontiguousarray(feature[b].T),
            "adjT": np.ascontiguousarray(adj[b].T),
            "blendW": blend_W, "blendb": blend_b,
        }
        for u, e in (("a", ea), ("b", eb)):
            pk = packs[e]
            m[f"W1cat_{u}"] = pk["W1cat"]; m[f"W1s_{u}"] = pk["W1s"]
            m[f"W2_{u}"] = pk["W2"]; m[f"w2sd_{u}"] = pk["w2sd"]
            m[f"colmask_{u}"] = pk["colmask"]
            if e < 0:
                rm = onesS; wm = np.zeros(S, np.float32); al = 1.0
            else:
                rm = rmask[b, :, e]; wm = rm; al = 0.0
            m[f"rmask_{u}"] = np.ascontiguousarray(rm.reshape(1, S))
            m[f"wmask_{u}"] = np.ascontiguousarray(wm.reshape(1, S))
            m[f"alpha_{u}"] = np.array([[al]], np.float32)
        in_maps.append(m)

    res = run_bass_kernel_spmd(nc, in_maps, core_ids=list(range(8)))

    final = np.empty((B, S, D), np.float32)
    bsum_total = np.float32(0.0)
    for b in range(B):
        final[b] = res.results[2 * b]["out"] + res.results[2 * b + 1]["out"]
        bsum_total += res.results[2 * b]["bsum"].sum(dtype=np.float32)
    mc = np.float32(bsum_total / (B * S * D))
    loss = np.float32(abs(mc - np.float32(TARGET_MAIN_CONTRIBUTION))
                      * np.float32(CONTRIB_LOSS_COEF))
    return final, loss, mc


# revision 29
# speedup vs baseline: 1.2498x; 1.1855x over previous
"""Trainium2 Bass kernel for nn_MoEGraphLayer (moe_routing).

Sharding: 16 GAT units (4 graphs x {main, dep0, dep1, dep2}) over 8 cores,
2 units per core.  core c: batch b = c//2; half = c%2.
  half 0: unit a = main GAT,  unit b = deputy 0 (sentence)
  half 1: unit a = deputy 1 (section), unit b = deputy 2 (doc)
Each core also computes the blend gate for its batch; core outputs a partial
final output [S, D]; host adds the two halves per batch.

All attention math runs transposed (pT[t,s]) so the aggregation matmul needs
no on-device transposes of [S,S] tensors; adjacency is host-transposed.
Router top-2 mask is computed host-side (it is part of the sharding decision
and needs exact fp32 compare semantics vs the reference).
"""
import numpy as np
from contextlib import ExitStack

import concourse.bass as bass
import concourse.mybir as mybir
import concourse.tile as tile
from concourse import bacc
from concourse.bass_utils import run_bass_kernel_spmd
from concourse.masks import make_identity

F32 = mybir.dt.float32
F32R = mybir.dt.float32r
BF16 = mybir.dt.bfloat16
FP8 = mybir.dt.float8e4
AF = mybir.ActivationFunctionType
ALU = mybir.AluOpType

B, S, D, H, Hd, E = 4, 1024, 256, 6, 64, 3
NT = S // 128          # 8 t-chunks
ND = D // 128          # 2 d-chunks
NK = (H * Hd) // 128   # 3 k-chunks for W2
BIG = 60.0
TOP_K = 2
TARGET_MAIN_CONTRIBUTION = 0.6
CONTRIB_LOSS_COEF = 0.01

_NC_CACHE = {}


def _build_program():
    nc = bacc.Bacc("TRN2", target_bir_lowering=False, debug=False, num_devices=8)

    # ---------------- DRAM I/O ----------------
    xT_d = nc.dram_tensor("xT", [D, S], F32, kind="ExternalInput").ap()
    adjT_d = nc.dram_tensor("adjT", [S, S], F32, kind="ExternalInput").ap()
    blendW_d = nc.dram_tensor("blendW", [D, D], F32, kind="ExternalInput").ap()
    blendb_d = nc.dram_tensor("blendb", [D, 1], F32, kind="ExternalInput").ap()
    UW = {}
    for u in ("a", "b"):
        UW[u] = dict(
            W1cat=nc.dram_tensor(f"W1cat_{u}", [D, H * Hd + H], F32, kind="ExternalInput").ap(),
            W1s=nc.dram_tensor(f"W1s_{u}", [D, H], F32, kind="ExternalInput").ap(),
            W2=nc.dram_tensor(f"W2_{u}", [H * Hd, D], F32, kind="ExternalInput").ap(),
            w2sd=nc.dram_tensor(f"w2sd_{u}", [H * Hd, 2], F32, kind="ExternalInput").ap(),
            colmask=nc.dram_tensor(f"colmask_{u}", [128, NT], F32, kind="ExternalInput").ap(),
            rmask=nc.dram_tensor(f"rmask_{u}", [1, S], F32, kind="ExternalInput").ap(),
            wmask=nc.dram_tensor(f"wmask_{u}", [1, S], F32, kind="ExternalInput").ap(),
            alpha=nc.dram_tensor(f"alpha_{u}", [1, 1], F32, kind="ExternalInput").ap(),
        )
    out_d = nc.dram_tensor("out", [S, D], F32, kind="ExternalOutput").ap()
    bsum_d = nc.dram_tensor("bsum", [128, ND], F32, kind="ExternalOutput").ap()

    with tile.TileContext(nc) as tc, ExitStack() as ctx:
        # ---------------- pools ----------------
        wpool = ctx.enter_context(tc.tile_pool(name="weights", bufs=1))
        base = ctx.enter_context(tc.tile_pool(name="base", bufs=1))
        work = ctx.enter_context(tc.tile_pool(name="work", bufs=3))
        ppool = ctx.enter_context(tc.tile_pool(name="ppool", bufs=3))
        bcast = ctx.enter_context(tc.tile_pool(name="bcast", bufs=2))
        pairp = ctx.enter_context(tc.tile_pool(name="pair", bufs=2))
        combp = ctx.enter_context(tc.tile_pool(name="comb", bufs=3))
        osbp = ctx.enter_context(tc.tile_pool(name="osb", bufs=4))
        rowp = ctx.enter_context(tc.tile_pool(name="rows", bufs=1))
        normp = ctx.enter_context(tc.tile_pool(name="norm", bufs=3))
        qpair = ctx.enter_context(tc.tile_pool(name="qpair", bufs=2))
        ubuf = {u: ctx.enter_context(tc.tile_pool(name=f"ubuf_{u}", bufs=1)) for u in "ab"}
        acc = ctx.enter_context(tc.tile_pool(name="acc", bufs=2, space="PSUM"))
        tmp = ctx.enter_context(tc.tile_pool(name="tmp", bufs=2, space="PSUM"))
        dr = ctx.enter_context(tc.tile_pool(name="dram", bufs=1, space="DRAM"))

        const1 = nc.const_aps.tensor(1.0, (128, 1), F32)

        # ---------------- constants / setup ----------------
        ident = base.tile([128, 128], F32)
        make_identity(nc, ident[:])
        onesr = base.tile([128, 1], BF16)
        nc.vector.tensor_copy(onesr[:], const1)
        negbig = base.tile([128, 1], F32)
        nc.vector.tensor_scalar(negbig[:], in0=const1, scalar1=-BIG, scalar2=None,
                                op0=ALU.mult)

        # ---------------- loads + setup, ordered by urgency ---------------
        # urgency: xT -> rmask/xTu -> W1s -> es chains -> cm/adjE -> W1cat ->
        # stage1 -> blend weights -> blend/w8 -> W2/w2sd (late)
        xTrr = base.tile([128, ND, S], F32R, tag="xTrr")  # raw feature^T (f32 bits)
        for j in range(ND):
            nc.sync.dma_start(xTrr[:, j, :], xT_d[bass.ts(j, 128), :].bitcast(F32R))

        W1cat = {}; W1s = {}; W2 = {}; w2sd = {}; cmask = {}; alpha = {}
        h65 = {}; ed = {}; h1e = {}; fg = {}; h2e = {}; outTn = {}
        esdr = {}; recdr = {}; fdr = {}
        for u in ("a", "b"):
            esdr[u] = dr.tile([H, S], F32, tag=f"esdr{u}", name=f"esdr{u}")
            recdr[u] = dr.tile([H, S], F32, tag=f"recdr{u}", name=f"recdr{u}")
            fdr[u] = dr.tile([1, S], F32, tag=f"fdr{u}", name=f"fdr{u}")

        xTu = {}
        for u in ("a", "b"):
            rmb = bcast.tile([128, S], F32, tag="bcast")
            nc.sync.dma_start(rmb[:], UW[u]["rmask"].to_broadcast((128, S)))
            xTu[u] = base.tile([128, ND, S], F32R, tag=f"xTu_{u}", name=f"xTu_{u}")
            nc.vector.tensor_mul(xTu[u][:], xTrr[:].bitcast(F32),
                                 rmb[:, None, :].broadcast_to((128, ND, S)))
            W1s[u] = wpool.tile([128, ND, H], F32R, tag=f"w1s{u}", name=f"w1s{u}")
            for j in range(ND):
                nc.sync.dma_start(W1s[u][:, j, :],
                                  UW[u]["W1s"][bass.ts(j, 128), :].bitcast(F32R))
            # es rows via W1s -> psum [H, S] -> sbuf -> dram (early: the DRAM
            # roundtrip overlaps everything that follows)
            esp = acc.tile([H, S], F32, tag="acc", name="esp")
            for nh in range(2):
                for j in range(ND):
                    nc.tensor.matmul(esp[:, bass.ts(nh, 512)], W1s[u][:, j, :],
                                     xTu[u][:, j, bass.ts(nh, 512)],
                                     start=(j == 0), stop=(j == ND - 1))
            essb = pairp.tile([H, S], F32, tag="pair", name="essb")
            nc.vector.tensor_copy(essb[:], esp[:])
            nc.sync.dma_start(esdr[u][:], essb[:])

        # adjacency (needed by L1(a) tau 0 early)
        for u in ("a", "b"):
            cmask[u] = wpool.tile([128, NT], F32, tag=f"cm{u}", name=f"cm{u}")
            nc.sync.dma_start(cmask[u][:], UW[u]["colmask"][:])
        W1cat["a"] = wpool.tile([128, ND, H * Hd + H], F32R, tag="w1ca", name="w1ca")
        for j in range(ND):
            nc.sync.dma_start(W1cat["a"][:, j, :],
                              UW["a"]["W1cat"][bass.ts(j, 128), :].bitcast(F32R))
        adjE = {u: base.tile([128, NT, S], FP8, tag=f"adjE_{u}", name=f"adjE_{u}") for u in "ab"}
        for t in range(NT):
            atmp = work.tile([128, S], F32, tag="work", name="atmp")
            nc.sync.dma_start(atmp[:], adjT_d[bass.ts(t, 128), :])
            for u in ("a", "b"):
                nc.gpsimd.tensor_scalar(adjE[u][:, t, :], in0=atmp[:],
                                        scalar1=cmask[u][:, t:t + 1], scalar2=None,
                                        op0=ALU.mult)
        W1cat["b"] = wpool.tile([128, ND, H * Hd + H], F32R, tag="w1cb", name="w1cb")
        for j in range(ND):
            nc.sync.dma_start(W1cat["b"][:, j, :],
                              UW["b"]["W1cat"][bass.ts(j, 128), :].bitcast(F32R))

        def stage1(u):
            # h_all + ed via W1cat;  h65 layout: per head 65 cols = [ones | h]
            h65[u] = base.tile([128, NT, H * 65], BF16, tag=f"h65_{u}", name=f"h65_{u}")
            ed[u] = base.tile([128, NT, H], F32, tag=f"ed_{u}", name=f"ed_{u}")
            for t in range(NT):
                m1p = tmp.tile([128, H * Hd + H], F32, tag="tmp", name="m1p")
                for j in range(ND):
                    nc.tensor.matmul(m1p[:], xTu[u][:, j, bass.ts(t, 128)],
                                     W1cat[u][:, j, :],
                                     start=(j == 0), stop=(j == ND - 1))
                dst = h65[u][:, t, :].rearrange("p (g c) -> p g c", g=H)
                nc.vector.tensor_copy(
                    dst[:, :, 1:65],
                    m1p[:, 0:H * Hd].rearrange("p (g c) -> p g c", g=H))
                nc.vector.tensor_copy(dst[:, :, 0:1], const1.to_broadcast((128, H, 1)))
                nc.vector.tensor_copy(ed[u][:, t, :], m1p[:, H * Hd:H * Hd + H])

        stage1("a")

        # blend gate + combine weights (off the early critical path)
        blendW = wpool.tile([128, ND, D], F32R, tag="bw")
        bb = wpool.tile([128, ND], F32, tag="bb")
        for j in range(ND):
            nc.sync.dma_start(blendW[:, j, :],
                              blendW_d[bass.ts(j, 128), :].bitcast(F32R))
            nc.sync.dma_start(bb[:, j:j + 1], blendb_d[bass.ts(j, 128), :])
        bs = base.tile([128, ND], F32)
        wmb = {}
        for u in ("a", "b"):
            alpha[u] = wpool.tile([128, 1], F32, tag=f"al{u}", name=f"al{u}")
            nc.sync.dma_start(alpha[u][:], UW[u]["alpha"].to_broadcast((128, 1)))
            wmb[u] = bcast.tile([128, S], F32, tag="bcast", name=f"wmb_{u}")
            nc.sync.dma_start(wmb[u][:], UW[u]["wmask"].to_broadcast((128, S)))
        w8 = {u: base.tile([128, ND, S], BF16, tag=f"w8_{u}", name=f"w8_{u}") for u in "ab"}
        for m in range(ND):
            bp = tmp.tile([128, S], F32, tag="tmp", name="bp")
            for nh in range(2):
                for j in range(ND):
                    nc.tensor.matmul(
                        bp[:, bass.ts(nh, 512)],
                        blendW[:, j, bass.ts(m, 128)],
                        xTrr[:, j, bass.ts(nh, 512)],
                        start=(j == 0), stop=(j == ND - 1))
            blm = combp.tile([128, S], F32, tag="comb", name="blm")
            nc.scalar.activation(blm[:], bp[:], AF.Sigmoid,
                                 bias=bb[:, m:m + 1], scale=1.0,
                                 accum_out=bs[:, m:m + 1])
            for u in ("a", "b"):
                t_ = combp.tile([128, S], F32, tag="comb", name="t_")
                nc.vector.scalar_tensor_tensor(
                    t_[:], in0=blm[:], scalar=1.0, in1=wmb[u][:],
                    op0=ALU.subtract, op1=ALU.mult)
                nc.vector.scalar_tensor_tensor(
                    w8[u][:, m, :], in0=blm[:], scalar=alpha[u][:],
                    in1=t_[:], op0=ALU.mult, op1=ALU.subtract)
        nc.sync.dma_start(bsum_d[:], bs[:])

        stage1("b")

        # late weights: W2 / w2sd (first needed after L1)
        for u in ("a", "b"):
            W2[u] = wpool.tile([128, NK, D], F32R, tag=f"w2{u}", name=f"w2{u}")
            w2sd[u] = wpool.tile([128, NK, 2], F32R, tag=f"w2sd{u}", name=f"w2sd{u}")
            for k in range(NK):
                nc.sync.dma_start(W2[u][:, k, :],
                                  UW[u]["W2"][bass.ts(k, 128), :].bitcast(F32R))
                nc.sync.dma_start(w2sd[u][:, k, :],
                                  UW[u]["w2sd"][bass.ts(k, 128), :].bitcast(F32R))

        # ---- per-unit pipeline: L1 attention -> normalize+ELU -> W2 + f/g ----
        # Unit-sequential so unit b's ACT-heavy L1 overlaps unit a's DVE/PE tail.
        h1raw = {u: ubuf[u].tile([128, NK, S], F32, tag=f"ubuf_{u}", name=f"h1raw_{u}") for u in "ab"}

        def normalize_chunk(u, k):
            """h1e[:, k, :] = elu(h1raw[:, k, :] * recip_pair) as f32r."""
            pair = pairp.tile([128, S], F32, tag="pair", name="pair")
            nc.sync.dma_start(pair[0:64, :],
                              recdr[u][2 * k:2 * k + 1, :].to_broadcast((64, S)))
            nc.sync.dma_start(pair[64:128, :],
                              recdr[u][2 * k + 1:2 * k + 2, :].to_broadcast((64, S)))
            t1 = normp.tile([128, S], F32, tag="norm", name="t1")
            nc.gpsimd.tensor_tensor(t1[:], in0=h1raw[u][:, k, :], in1=pair[:],
                                    op=ALU.mult)
            mn = normp.tile([128, S], F32, tag="norm", name="mn")
            nc.vector.tensor_scalar(mn[:], in0=t1[:], scalar1=0.0, scalar2=None,
                                    op0=ALU.min)
            em = normp.tile([128, S], F32, tag="norm", name="em")
            nc.scalar.activation(em[:], mn[:], AF.Exp)
            r = normp.tile([128, S], F32, tag="norm", name="r")
            nc.scalar.activation(r[:], t1[:], AF.Relu)
            nc.vector.scalar_tensor_tensor(
                h1e[u][:, k, :], in0=em[:], scalar=-1.0, in1=r[:],
                op0=ALU.add, op1=ALU.add)

        for u in ("a", "b"):
            h1e[u] = base.tile([128, NK, S], F32R, tag=f"xTu_{u}", name=f"h1e_{u}")
            for hp in range(H // 2):
                h0 = 2 * hp
                esb = {}
                for g in (0, 1):
                    esb[g] = bcast.tile([128, S], F32, tag="bcast", name="esb")
                    nc.sync.dma_start(
                        esb[g][:], esdr[u][h0 + g:h0 + g + 1, :].to_broadcast((128, S)))
                h1p = {g: acc.tile([65, S], F32, tag="acc", name="h1p") for g in (0, 1)}
                for t in range(NT):
                    qp = qpair.tile([128, 2, S], F32, tag="qpair", name="qp")
                    for g in (0, 1):
                        e = work.tile([128, S], F32, tag="work", name="e")
                        nc.scalar.activation(e[:], esb[g][:], AF.Prelu,
                                             bias=ed[u][:, t, h0 + g:h0 + g + 1],
                                             scale=1.0, alpha=0.2)
                        nc.vector.scalar_tensor_tensor(
                            qp[:, g, :], in0=e[:], scalar=BIG, in1=adjE[u][:, t, :],
                            op0=ALU.add, op1=ALU.mult)
                    pp = ppool.tile([128, 2, S], BF16, tag="ppool", name="pp")
                    nc.scalar.activation(pp[:], qp[:], AF.Exp, bias=negbig[:], scale=1.0)
                    for g in (0, 1):
                        for nh in range(2):
                            nc.tensor.matmul(
                                h1p[g][:, bass.ts(nh, 512)],
                                h65[u][:, t, (h0 + g) * 65:(h0 + g + 1) * 65],
                                pp[:, g, bass.ts(nh, 512)],
                                start=(t == 0), stop=(t == NT - 1))
                for g in (0, 1):
                    # rowsum (row 0) -> recip -> dram; h rows (1:65) -> h1raw
                    rrow = rowp.tile([1, S], F32, tag="rows", name="rrow")
                    nc.vector.reciprocal_approx_fast(rrow[:], h1p[g][0:1, :])
                    nc.sync.dma_start(recdr[u][h0 + g:h0 + g + 1, :], rrow[:])
                    stg = pairp.tile([65, S], F32, tag="pair", name="stg")
                    nc.vector.tensor_copy(stg[:], h1p[g][:])
                    nc.sync.dma_start(
                        h1raw[u][g * 64:g * 64 + 64, hp, :], stg[1:65, :])
                if hp >= 1:
                    normalize_chunk(u, hp - 1)
            normalize_chunk(u, 2)

            # W2 projection + f/g columns
            h2e[u] = ubuf[u].tile([128, NT, 2, 128], BF16, tag=f"ubuf_{u}", name=f"h2e_{u}")
            fg[u] = base.tile([128, NT, 2], F32, tag=f"fg_{u}", name=f"fg_{u}")
            for t in range(NT):
                w2p = tmp.tile([128, D], F32, tag="tmp", name="w2p")
                for k in range(NK):
                    nc.tensor.matmul(w2p[:], h1e[u][:, k, bass.ts(t, 128)],
                                     W2[u][:, k, :], start=(k == 0), stop=(k == NK - 1))
                nc.vector.tensor_copy(
                    h2e[u][:, t, :, :],
                    w2p[:].rearrange("p (m c) -> p m c", m=2))
                fgp = tmp.tile([128, 2], F32, tag="tmp", name="fgp")
                for k in range(NK):
                    nc.tensor.matmul(fgp[:], h1e[u][:, k, bass.ts(t, 128)],
                                     w2sd[u][:, k, :], start=(k == 0), stop=(k == NK - 1))
                nc.vector.tensor_copy(fg[u][:, t, :], fgp[:])
                nc.sync.dma_start(fdr[u][0:1, bass.ts(t, 128)], fg[u][:, t, 0:1])

        # ---- L2 attention + fused normalize*combine-weight -------------------
        for u in ("a", "b"):
            fb = bcast.tile([128, S], F32, tag="bcast", name="fb")
            nc.sync.dma_start(fb[:], fdr[u][:].to_broadcast((128, S)))
            aggA = acc.tile([128, S], F32, tag="acc", name="aggA")
            aggB = acc.tile([128, S], F32, tag="acc", name="aggB")
            rs2 = tmp.tile([1, S], F32, tag="tmp", name="rs2")
            for t in range(NT):
                e2 = work.tile([128, S], F32, tag="work", name="e2")
                nc.scalar.activation(e2[:], fb[:], AF.Prelu,
                                     bias=fg[u][:, t, 1:2], scale=1.0, alpha=0.2)
                q2 = work.tile([128, S], F32, tag="work", name="q2")
                nc.vector.scalar_tensor_tensor(
                    q2[:], in0=e2[:], scalar=BIG, in1=adjE[u][:, t, :],
                    op0=ALU.add, op1=ALU.mult)
                p2 = ppool.tile([128, S], BF16, tag="ppool", name="p2")
                nc.scalar.activation(p2[:], q2[:], AF.Exp, bias=negbig[:], scale=1.0)
                for nh in range(2):
                    nc.tensor.matmul(aggA[:, bass.ts(nh, 512)], h2e[u][:, t, 0, :],
                                     p2[:, bass.ts(nh, 512)],
                                     start=(t == 0), stop=(t == NT - 1))
                    nc.tensor.matmul(aggB[:, bass.ts(nh, 512)], h2e[u][:, t, 1, :],
                                     p2[:, bass.ts(nh, 512)],
                                     start=(t == 0), stop=(t == NT - 1))
                    nc.tensor.matmul(rs2[:, bass.ts(nh, 512)], onesr[:],
                                     p2[:, bass.ts(nh, 512)],
                                     start=(t == 0), stop=(t == NT - 1))
            r2row = rowp.tile([1, S], F32, tag="rows", name="r2row")
            nc.vector.reciprocal_approx_fast(r2row[:], rs2[:])
            r2b = pairp.tile([128, S], F32, tag="pair", name="r2b")
            nc.gpsimd.partition_broadcast(r2b[:], r2row[0:1, :])
            # unit a reuses the (long-dead) xTrr slot
            outTn[u] = base.tile([128, ND, S], F32, name=f"outTn_{u}",
                                 tag=("xTrr" if u == "a" else f"outTn_{u}"))
            for m, aggM in ((0, aggA), (1, aggB)):
                rw = combp.tile([128, S], F32, tag="comb", name="rw")
                nc.vector.tensor_mul(rw[:], r2b[:], w8[u][:, m, :])
                nc.vector.tensor_mul(outTn[u][:, m, :], aggM[:], rw[:])

        # ---- final add + transpose out (block-wise, pipelined) ----------------
        for m in range(ND):
            for sgm in range(NT):
                pb = combp.tile([128, 128], F32, tag="comb", name="pblk")
                nc.vector.tensor_add(pb[:],
                                     outTn["a"][:, m, bass.ts(sgm, 128)],
                                     outTn["b"][:, m, bass.ts(sgm, 128)])
                tp = tmp.tile([128, 128], F32, tag="tmp", name="tp")
                nc.tensor.transpose(tp[:], pb[:], ident[:])
                osb = osbp.tile([128, 128], F32, tag="osb", name="osb")
                nc.vector.tensor_copy(osb[:], tp[:])
                nc.sync.dma_start(
                    out_d[bass.ts(sgm, 128), bass.ts(m, 128)], osb[:])

    nc.compile()
    return nc


def _get_runner():
    """Build (once) a persistent jitted 8-core runner for the program.

    Mirrors bass2jax.run_bass_via_pjrt's multi-core path, but caches the
    jitted shard_map callable so repeated kernel() calls skip re-tracing
    and re-compiling (~0.5 s/call)."""
    if "runner" in _NC_CACHE:
        return _NC_CACHE["runner"]
    import jax
    from jax.experimental.shard_map import shard_map
    from jax.sharding import Mesh, PartitionSpec
    from concourse import bass2jax

    nc = _NC_CACHE.get("nc")
    if nc is None:
        nc = _build_program()
        _NC_CACHE["nc"] = nc
    bass2jax.install_neuronx_cc_hook()
    assert nc.dbg_addr is None
    pname = nc.partition_id_tensor.name if nc.partition_id_tensor else None

    in_names, out_names, out_avals, zero_outs = [], [], [], []
    for alloc in nc.m.functions[0].allocations:
        if not isinstance(alloc, mybir.MemoryLocationSet):
            continue
        name = alloc.memorylocations[0].name
        if alloc.kind == "ExternalInput":
            if name != pname:
                in_names.append(name)
        elif alloc.kind == "ExternalOutput":
            shape = tuple(alloc.tensor_shape)
            dtype = mybir.dt.np(alloc.dtype)
            out_names.append(name)
            out_avals.append(jax.core.ShapedArray(shape, dtype))
            zero_outs.append(np.zeros(shape, dtype))
    n_params = len(in_names)
    all_names = in_names + out_names
    if pname is not None:
        all_names = all_names + [pname]
    donate = tuple(range(n_params, n_params + len(out_names)))

    def _body(*args):
        operands = list(args)
        if pname is not None:
            operands.append(bass2jax.partition_id_tensor())
        outs = bass2jax._bass_exec_p.bind(
            *operands,
            out_avals=tuple(out_avals),
            in_names=tuple(all_names),
            out_names=tuple(out_names),
            lowering_input_output_aliases=(),
            sim_require_finite=True,
            sim_require_nnan=True,
            nc=nc,
        )
        return tuple(outs)

    devices = jax.devices()[:8]
    mesh = Mesh(np.asarray(devices), ("core",))
    nio = n_params + len(out_names)
    sharded = jax.jit(
        shard_map(_body, mesh=mesh, in_specs=(PartitionSpec("core"),) * nio,
                  out_specs=(PartitionSpec("core"),) * len(out_names),
                  check_rep=False),
        donate_argnums=donate, keep_unused=True)
    runner = (sharded, in_names, out_names, out_avals, zero_outs)
    _NC_CACHE["runner"] = runner
    return runner


def _run_spmd(in_maps):
    sharded, in_names, out_names, out_avals, zero_outs = _get_runner()
    n_cores = len(in_maps)
    concat_in = [
        np.concatenate([np.asarray(in_maps[c][name]) for c in range(n_cores)], axis=0)
        for name in in_names
    ]
    concat_zeros = [
        np.zeros((n_cores * z.shape[0], *z.shape[1:]), z.dtype) for z in zero_outs
    ]
    out_arrs = sharded(*concat_in, *concat_zeros)
    return [
        {name: np.asarray(out_arrs[i]).reshape(n_cores, *out_avals[i].shape)[c]
         for i, name in enumerate(out_names)}
        for c in range(n_cores)
    ]


def _routing_masks(feature, router_W):
    """Top-2-of-3 routing masks, replicating jax.lax.top_k tie semantics."""
    x = feature.reshape(-1, D).astype(np.float32)
    logits = x @ router_W.astype(np.float32)
    m = logits.max(axis=-1, keepdims=True)
    ex = np.exp(logits - m, dtype=np.float32)
    g = ex / ex.sum(axis=-1, keepdims=True, dtype=np.float32)
    g0, g1, g2 = g[:, 0], g[:, 1], g[:, 2]
    drop2 = (g2 <= g0) & (g2 <= g1)
    drop1 = ~drop2 & (g1 <= g0) & (g1 <= g2)
    drop0 = ~(drop1 | drop2)
    mask = np.stack([~drop0, ~drop1, ~drop2], axis=-1).astype(np.float32)
    return mask.reshape(B, S, E)


def kernel(feature, adj, main_W1, main_a1s, main_a1d, main_W2, main_a2s, main_a2d,
           dep_W1, dep_a1s, dep_a1d, dep_W2, dep_a2s, dep_a2d,
           router_W, blend_W, blend_b, doc_num, sect_num):
    feature = np.asarray(feature, dtype=np.float32)
    adj = np.asarray(adj, dtype=np.float32)
    doc_num = int(doc_num)
    sect_num = int(sect_num)


    # ---- host-side prep (sharding + weight packing) ----
    col = np.arange(S)
    colmasks = [
        (col < S - sect_num - doc_num).astype(np.float32),
        ((col >= S - sect_num - doc_num) & (col < S - doc_num)).astype(np.float32),
        (col >= S - doc_num).astype(np.float32),
    ]
    rmask = _routing_masks(feature, router_W)      # [B, S, E]
    onesS = np.ones(S, np.float32)

    def pack_unit(expert):
        """expert: -1 = main, 0..2 = deputy index. Returns weight dict pieces."""
        if expert < 0:
            W1, a1s, a1d = main_W1, main_a1s, main_a1d
            W2_, a2s, a2d = main_W2, main_a2s, main_a2d
            cm = onesS
        else:
            W1, a1s, a1d = dep_W1[expert], dep_a1s[expert], dep_a1d[expert]
            W2_, a2s, a2d = dep_W2[expert], dep_a2s[expert], dep_a2d[expert]
            cm = colmasks[expert]
        W1 = np.asarray(W1, np.float32); W2_ = np.asarray(W2_, np.float32)
        a1s = np.asarray(a1s, np.float32); a1d = np.asarray(a1d, np.float32)
        a2s = np.asarray(a2s, np.float32); a2d = np.asarray(a2d, np.float32)
        W1cat = np.concatenate(
            [W1.transpose(1, 0, 2).reshape(D, H * Hd),
             np.einsum("hdf,hf->dh", W1, a1d)], axis=1)          # [D, 390]
        W1s_ = np.einsum("hdf,hf->dh", W1, a1s)                   # [D, 6]
        w2sd = np.stack([W2_ @ a2s, W2_ @ a2d], axis=1)           # [384, 2]
        return dict(W1cat=np.ascontiguousarray(W1cat),
                    W1s=np.ascontiguousarray(W1s_),
                    W2=np.ascontiguousarray(W2_),
                    w2sd=np.ascontiguousarray(w2sd),
                    colmask=np.ascontiguousarray(cm.reshape(NT, 128).T))

    unit_specs = {0: (-1, 0), 1: (1, 2)}  # half -> (expert_a, expert_b)
    packs = {e: pack_unit(e) for e in (-1, 0, 1, 2)}
    blend_W = np.ascontiguousarray(np.asarray(blend_W, np.float32))
    blend_b = np.ascontiguousarray(np.asarray(blend_b, np.float32).reshape(D, 1))

    in_maps = []
    for c in range(8):
        b, half = c // 2, c % 2
        ea, eb = unit_specs[half]
        m = {
            "xT": np.ascontiguousarray(feature[b].T),
            "adjT": np.ascontiguousarray(adj[b].T),
            "blendW": blend_W, "blendb": blend_b,
        }
        for u, e in (("a", ea), ("b", eb)):
            pk = packs[e]
            m[f"W1cat_{u}"] = pk["W1cat"]; m[f"W1s_{u}"] = pk["W1s"]
            m[f"W2_{u}"] = pk["W2"]; m[f"w2sd_{u}"] = pk["w2sd"]
            m[f"colmask_{u}"] = pk["colmask"]
            if e < 0:
                rm = onesS; wm = np.zeros(S, np.float32); al = 1.0
            else:
                rm = rmask[b, :, e]; wm = rm; al = 0.0
            m[f"rmask_{u}"] = np.ascontiguousarray(rm.reshape(1, S))
            m[f"wmask_{u}"] = np.ascontiguousarray(wm.reshape(1, S))
            m[f"alpha_{u}"] = np.array([[al]], np.float32)
        in_maps.append(m)

    results = _run_spmd(in_maps)

    final = np.empty((B, S, D), np.float32)
    bsum_total = np.float32(0.0)
    for b in range(B):
        final[b] = results[2 * b]["out"] + results[2 * b + 1]["out"]
        bsum_total += results[2 * b]["bsum"].sum(dtype=np.float32)
    mc = np.float32(bsum_total / (B * S * D))
    loss = np.float32(abs(mc - np.float32(TARGET_MAIN_CONTRIBUTION))
                      * np.float32(CONTRIB_LOSS_COEF))
    return final, loss, mc
